# revision 1
# baseline (speedup 1.0000x reference)
"""Trainium2 Bass kernel for a Lorentz RGCN message-passing layer.

Strategy (8 NeuronCores, SPMD):
  - Nodes are range-partitioned: core c owns 6272 destination nodes
    (49 windows x 128).  All edges whose dst falls in a core's range are
    processed by that core, so no cross-core reduction is needed; each
    core writes a disjoint slice of the output.
  - Since NUM_BASES == D (SI=SO=1), the per-edge relation transform is
    elementwise: msg = h_tangent[src] * weight[etype] + rel_emb[etype].
  - The per-edge reduce weight w=norm[dst] is constant within a dst
    segment, so mu[n] = norm[n]/(norm[n]*deg(n)+1e-6) * sum_e msg_l[e];
    only a plain segment sum of [x0, xi, 1] (130 features) is needed.
  - Each core builds the full h_tangent table (rolled so its own nodes
    are rows [0, 6272)), then per-edge gathers via gpsimd.dma_gather:
    h rows are fetched in PAIRS (idx = rotated_src >> 1 < 25088, fits the
    gather's int16 index ucode) and the right half is selected by parity;
    weight/rel_emb come from a combined [230, 256] table.
  - Segment sums: edges are bucketed on the host into their dst window
    (128 nodes); each 128-edge tile is reduced on the TensorEngine with a
    one-hot selection matrix (scaled by the per-edge centroid weight) into
    a PSUM accumulator [128 nodes x 130].
  - Per-node epilogue (centroid normalization, log0, self-loop matmul,
    exp0) runs on 49 windows of 128 nodes with batched per-node scalars.
"""

import sys

sys.path.insert(0, "/opt/trn_rl_repo")

import numpy as np

import concourse.bass as bass
import concourse.bacc as bacc
import concourse.mybir as mybir
from concourse.tile import TileContext
from concourse.masks import make_identity
from concourse import library_config

# ---------------------------------------------------------------- constants
NCORES = 8
N = 50000
E = 800000
D = 128
R = 230
C = 0.01
SC = 0.1  # sqrt(C)
EPS = 1e-7

NPC = 6272                 # nodes per core = 49 windows * 128
NW = 49                    # windows per core
TPW = 18                   # tile budget per window (Poisson(2048) + 5.7 sigma)
EPW = TPW * 128            # 2304 edge slots per window
NTILES = NW * TPW          # 882
ESLOT = NTILES * 128       # 112896 edge slots per core
NROT = NCORES * NPC        # 50176 rows in the (rolled, padded) h table
CH_T = 9                   # tiles per gather chunk (half window)
CH_E = CH_T * 128          # 1152 edges per chunk
NCH = NTILES // CH_T       # 98 chunks
IDXC = ESLOT // 16         # 7056 index columns

f32 = mybir.dt.float32
i16 = mybir.dt.int16
i32 = mybir.dt.int32
i8 = mybir.dt.int8
OP = mybir.AluOpType
AF = mybir.ActivationFunctionType


# ------------------------------------------------------------ drain patch
def _patch_tile_drain():
    """This container's walrus build rejects instructions with more than one
    sync-wait; split the Tile tail drain's waits across multiple drains."""
    import bass_rust as _br
    from concourse.vector_clock import ScopedClock

    if getattr(TileContext, "_drain_patched", False):
        return

    def _patched(self, tick_clock, wait_clock):
        drain_inst = self.nc.sync.drain()
        wait_clock.add_sem_waits(
            drain_inst.ins, ScopedClock({None: tick_clock.global_clock})
        )
        si = drain_inst.ins.sync_info
        if si is not None and si.on_wait is not None and len(si.on_wait) > 1:
            waits = list(si.on_wait)
            ups = list(si.on_update) if si.on_update else []
            drain_inst.ins.sync_info = _br.SyncInfo(on_wait=waits[:1], on_update=ups)
            for w in waits[1:]:
                d2 = self.nc.sync.drain()
                d2.ins.sync_info = _br.SyncInfo(on_wait=[w], on_update=[])
        self.nc.all_engine_barrier()
        assert self.sems is not None
        popped = self.nc._tile_sem_poison_stack.pop()
        assert popped is self._sem_poison
        self.nc.clear_and_free_semaphores(list(self.sems.allocated().values()))
        self.nc.all_engine_barrier()

    TileContext._drain_and_barrier = _patched
    TileContext._drain_patched = True


def _split_multi_waits(nc):
    """This walrus build only encodes ONE sync-wait per instruction
    (NEURON_ISA_TPB_EVENTS has a single wait slot).  Tile's wait assignment
    can attach several; hoist the extras onto same-engine NoOps inserted
    immediately before the instruction."""
    import bass_rust as _br

    uid = [0]
    for f in nc.m.functions:
        for bb in f.blocks:
            insts = bb.instructions
            out = []
            changed = False
            for ins in insts:
                si = ins.sync_info
                if si is not None and si.on_wait is not None and len(si.on_wait) > 1:
                    waits = list(si.on_wait)
                    ups = list(si.on_update) if si.on_update else []
                    for w in waits[:-1]:
                        uid[0] += 1
                        nop = mybir.InstNoOp(
                            name=f"waitsplit-{uid[0]}", ins=[], outs=[]
                        )
                        nop.engine = ins.engine
                        nop.sync_info = _br.SyncInfo(on_wait=[w], on_update=[])
                        nc.register_instruction(nop, overwrite=True)
                        out.append(nop)
                    ins.sync_info = _br.SyncInfo(on_wait=[waits[-1]], on_update=ups)
                    changed = True
                out.append(ins)
            if changed:
                bb.instructions = out


# ------------------------------------------------------------ device program
_PROGRAM = None


def _build_program():
    import os
    stage = os.environ.get("KSTAGE", "full")
    nc = bacc.Bacc("TRN2", target_bir_lowering=False, debug=False)

    h_roll = nc.declare_dram_parameter("h_roll", [NROT, D], f32, isOutput=False)
    wr_tab = nc.declare_dram_parameter("wr_tab", [R, 2 * D], f32, isOutput=False)
    lw_d = nc.declare_dram_parameter("lw", [D, D], f32, isOutput=False)
    ev_d = nc.declare_dram_parameter("ev", [D, D], f32, isOutput=False)
    norm_d = nc.declare_dram_parameter("norm_c", [NPC, 1], f32, isOutput=False)
    idxh_d = nc.declare_dram_parameter("idx_h", [128, IDXC], i16, isOutput=False)
    idxw_d = nc.declare_dram_parameter("idx_wr", [128, IDXC], i16, isOutput=False)
    par_d = nc.declare_dram_parameter("par", [128, NTILES], i8, isOutput=False)
    drel_d = nc.declare_dram_parameter("drel", [128, NTILES], f32, isOutput=False)
    out_d = nc.declare_dram_parameter("out", [NPC, D], f32, isOutput=True)
    htab = nc.dram_tensor("htab", [NROT, D], f32)

    with TileContext(nc) as tc:
        with (
            tc.tile_pool(name="persist", bufs=1) as pp,
            tc.tile_pool(name="consts", bufs=1) as cp,
        ):
            S_all = pp.tile([128, NW, 130], f32)
            h_loc = pp.tile([128, NW, D], f32)
            hn = pp.tile([128, NW, D], f32)
            par_sb = pp.tile([128, NTILES], i8)
            drel_sb = pp.tile([128, NTILES], f32)
            norm_sb = pp.tile([128, NW], f32)
            s2raw = pp.tile([128, NW], f32)
            ne2 = pp.tile([128, NW], f32)

            LW = cp.tile([128, D], f32)
            EV = cp.tile([128, D], f32)
            iota_f = cp.tile([128, 128], f32)
            ident = cp.tile([128, 128], f32)
            iota_i = cp.tile([128, 128], i32)

            nc.sync.dma_start(out=par_sb[:], in_=par_d[:])
            nc.sync.dma_start(out=drel_sb[:], in_=drel_d[:])
            nc.sync.dma_start(
                out=norm_sb[:], in_=norm_d[:].rearrange("(w p) o -> p (w o)", p=128)
            )
            nc.sync.dma_start(out=LW[:], in_=lw_d[:])
            nc.sync.dma_start(out=EV[:], in_=ev_d[:])
            nc.gpsimd.iota(iota_i[:], pattern=[[1, 128]], base=0, channel_multiplier=0)
            nc.vector.tensor_copy(out=iota_f[:], in_=iota_i[:])
            make_identity(nc, ident[:])

            # ---------------- phase A: h_tangent table (log0 of h_roll) ----
            SUP = 14
            NSUP = NROT // (SUP * 128)  # 28
            with tc.tile_pool(name="phA", bufs=3) as pa:
                for s in range(NSUP):
                    xin = pa.tile([128, SUP, D], f32, tag="xin")
                    nc.sync.dma_start(
                        out=xin[:],
                        in_=h_roll[s * SUP * 128 : (s + 1) * SUP * 128, :].rearrange(
                            "(t p) d -> p t d", p=128
                        ),
                    )
                    n2 = pa.tile([128, SUP], f32, tag="n2")
                    for t in range(SUP):
                        sq = pa.tile([128, D], f32, tag="sq")
                        nc.scalar.activation(
                            sq[:], xin[:, t, :], AF.Square,
                            accum_out=n2[:, t : t + 1],
                        )
                    nr = pa.tile([128, SUP], f32, tag="nr")
                    nc.scalar.activation(nr[:], n2[:], AF.Sqrt)
                    nn = pa.tile([128, SUP], f32, tag="nn")
                    nc.vector.tensor_scalar(
                        out=nn[:], in0=nr[:], scalar1=EPS, scalar2=None, op0=OP.max
                    )
                    v = pa.tile([128, SUP], f32, tag="v")
                    nc.vector.tensor_scalar(
                        out=v[:], in0=nn[:], scalar1=SC, scalar2=1.0 - EPS,
                        op0=OP.mult, op1=OP.min,
                    )
                    la = pa.tile([128, SUP], f32, tag="la")
                    nc.scalar.activation(la[:], v[:], AF.Ln, bias=1.0, scale=1.0)
                    lb = pa.tile([128, SUP], f32, tag="lb")
                    nc.scalar.activation(lb[:], v[:], AF.Ln, bias=1.0, scale=-1.0)
                    df = pa.tile([128, SUP], f32, tag="df")
                    nc.vector.tensor_tensor(
                        out=df[:], in0=la[:], in1=lb[:], op=OP.subtract
                    )
                    rn = pa.tile([128, SUP], f32, tag="rn")
                    nc.vector.reciprocal(rn[:], nn[:])
                    sc1 = pa.tile([128, SUP], f32, tag="sc1")
                    nc.vector.tensor_tensor(
                        out=sc1[:], in0=df[:], in1=rn[:], op=OP.mult
                    )
                    scl = pa.tile([128, SUP], f32, tag="scl")
                    nc.vector.tensor_scalar(
                        out=scl[:], in0=sc1[:], scalar1=0.5 / SC, scalar2=None,
                        op0=OP.mult,
                    )
                    hts = pa.tile([128, SUP, D], f32, tag="hts")
                    scl_bc = bass.AP(
                        scl.tensor, scl.offset, [scl.ap[0], scl.ap[1], [0, D]]
                    )
                    nc.vector.tensor_tensor(
                        out=hts[:], in0=xin[:], in1=scl_bc, op=OP.mult
                    )
                    nc.sync.dma_start(
                        out=htab[s * SUP * 128 : (s + 1) * SUP * 128, :].rearrange(
                            "(t p) d -> p t d", p=128
                        ),
                        in_=hts[:],
                    )
                    for t in range(SUP):
                        g = s * SUP + t
                        if g < NW:
                            nc.scalar.copy(h_loc[:, g, :], hts[:, t, :])

            tc.strict_bb_all_engine_barrier()
            if stage == "A":
                nc.sync.dma_start(
                    out=out_d[:].rearrange("(w p) d -> p w d", p=128), in_=h_loc[:]
                )

            # ---------------- phase B/C: edges + per-window epilogue -------
            if stage == "A":
                return nc
            htab_pairs = htab[:].rearrange("(a b) d -> a (b d)", b=2)
            nreg = nc.gpsimd.to_reg(CH_E)
            nw_run = 1 if stage == "B1" else NW
            with (
                tc.tile_pool(name="phB", bufs=2) as pb,
                tc.tile_pool(name="chain", bufs=2) as pc,
                tc.tile_pool(name="scr", bufs=3) as scr,
                tc.tile_pool(name="psum", bufs=2, space="PSUM") as psp,
            ):
                for w in range(nw_run):
                    ps = psp.tile([128, 130], f32, tag="ps")
                    for half in range(2):
                        k = w * 2 + half
                        idxh_t = pb.tile([128, 72], i16, tag="idxh")
                        nc.sync.dma_start(
                            out=idxh_t[:], in_=idxh_d[:, 72 * k : 72 * (k + 1)]
                        )
                        idxw_t = pb.tile([128, 72], i16, tag="idxw")
                        nc.sync.dma_start(
                            out=idxw_t[:], in_=idxw_d[:, 72 * k : 72 * (k + 1)]
                        )
                        hb = pb.tile([128, CH_T, 2 * D], f32, tag="hb")
                        nc.gpsimd.dma_gather(
                            out_ap=hb[:], in_ap=htab_pairs, idxs_ap=idxh_t[:],
                            num_idxs=CH_E, num_idxs_reg=nreg, elem_size=2 * D,
                            single_packet=False,
                        )
                        wrb = pb.tile([128, CH_T, 2 * D], f32, tag="wrb")
                        nc.gpsimd.dma_gather(
                            out_ap=wrb[:], in_ap=wr_tab[:], idxs_ap=idxw_t[:],
                            num_idxs=CH_E, num_idxs_reg=nreg, elem_size=2 * D,
                            single_packet=False,
                        )
                        if stage == "B1g":
                            if half == 1:
                                nc.sync.dma_start(
                                    out=out_d[0:CH_E, :].rearrange(
                                        "(t p) d -> p t d", p=128
                                    ),
                                    in_=hb[:, :, 0:128],
                                )
                            continue
                        rhs = pb.tile([128, CH_T, 130], f32, tag="rhs")
                        msg = rhs[:, :, 0:128]
                        # select h row by parity of src
                        nc.scalar.copy(msg, hb[:, :, 0:128])
                        nc.vector.copy_predicated(
                            out=msg,
                            mask=par_sb[:, CH_T * k : CH_T * (k + 1)].to_broadcast(
                                [128, CH_T, 128]
                            ),
                            data=hb[:, :, 128:256],
                        )
                        nc.vector.tensor_tensor(
                            out=msg, in0=msg, in1=wrb[:, :, 0:128], op=OP.mult
                        )
                        nc.vector.tensor_tensor(
                            out=msg, in0=msg, in1=wrb[:, :, 128:256], op=OP.add
                        )
                        # per-edge norms (per 128-edge tile)
                        n2 = pc.tile([128, CH_T], f32, tag="n2")
                        for t in range(CH_T):
                            sq = scr.tile([128, D], f32, tag="sq")
                            nc.scalar.activation(
                                sq[:], rhs[:, t, 0:128], AF.Square,
                                accum_out=n2[:, t : t + 1],
                            )
                        # exp0 + to_lorentz scalar chain (per edge)
                        def TS(dst, src, s1, s2=None, o0=OP.mult, o1=None):
                            if o1 is None:
                                nc.vector.tensor_scalar(
                                    out=dst, in0=src, scalar1=s1, scalar2=None, op0=o0
                                )
                            else:
                                nc.vector.tensor_scalar(
                                    out=dst, in0=src, scalar1=s1, scalar2=s2,
                                    op0=o0, op1=o1,
                                )

                        def TT(dst, a, b, op):
                            nc.vector.tensor_tensor(out=dst, in0=a, in1=b, op=op)

                        def PCT(tag):
                            return pc.tile([128, CH_T], f32, tag=tag, name=tag)[:]

                        nraw = PCT("nraw")
                        nc.scalar.activation(nraw, n2[:], AF.Sqrt)
                        nn = PCT("nn")
                        TS(nn, nraw, EPS, o0=OP.max)
                        th = PCT("th")
                        nc.scalar.activation(th, nn, AF.Tanh, scale=SC)
                        t2 = PCT("t2")
                        TT(t2, th, th, OP.mult)
                        dn = PCT("dn")
                        TS(dn, t2, -1.0, 1.0, OP.mult, OP.add)
                        dnc = PCT("dnc")
                        TS(dnc, dn, EPS, o0=OP.max)
                        rd = PCT("rd")
                        nc.vector.reciprocal(rd, dnc)
                        rn = PCT("rn")
                        nc.vector.reciprocal(rn, nn)
                        q1 = PCT("q1")
                        TT(q1, th, rn, OP.mult)
                        q2 = PCT("q2")
                        TT(q2, q1, rd, OP.mult)
                        sxi = PCT("sxi")
                        TS(sxi, q2, 2.0 / SC)
                        inv = PCT("inv")
                        nc.vector.reciprocal(inv, sxi)
                        q3 = PCT("q3")
                        TS(q3, t2, 1.0, 1.0 / SC, OP.add, OP.mult)
                        x0 = PCT("x0")
                        TT(x0, q3, rd, OP.mult)
                        x0p = PCT("x0p")
                        TT(x0p, x0, inv, OP.mult)
                        nc.scalar.copy(rhs[:, :, 128], x0p)
                        nc.scalar.copy(rhs[:, :, 129], inv)
                        # selection matrices + segment-sum matmuls
                        selc = scr.tile([128, CH_T, 128], f32, tag="selc")
                        iota_bc = bass.AP(
                            iota_f.tensor, iota_f.offset,
                            [iota_f.ap[0], [0, CH_T], iota_f.ap[1]],
                        )
                        drel_sl = drel_sb[:, CH_T * k : CH_T * (k + 1)]
                        drel_bc = bass.AP(
                            drel_sl.tensor, drel_sl.offset,
                            [drel_sl.ap[0], drel_sl.ap[1], [0, 128]],
                        )
                        nc.vector.tensor_tensor(
                            out=selc[:], in0=iota_bc, in1=drel_bc, op=OP.is_equal
                        )
                        sxi_bc = bass.AP(
                            sxi.tensor, sxi.offset,
                            [sxi.ap[0], sxi.ap[1], [0, 128]],
                        )
                        nc.vector.tensor_tensor(
                            out=selc[:], in0=selc[:], in1=sxi_bc, op=OP.mult
                        )
                        for t in range(CH_T):
                            nc.tensor.matmul(
                                ps[:], selc[:, t, :], rhs[:, t, :],
                                start=(half == 0 and t == 0),
                                stop=(half == 1 and t == CH_T - 1),
                            )
                    # ---------------- phase C (per window) -----------------
                    if stage == "B1g":
                        return nc
                    nc.scalar.copy(S_all[:, w, :], ps[:])
                    sq2 = scr.tile([128, 129], f32, tag="sq2")
                    nc.scalar.activation(
                        sq2[:], S_all[:, w, 0:129], AF.Square,
                        accum_out=s2raw[:, w : w + 1],
                    )
                    tp = psp.tile([128, 128], f32, tag="tp")
                    nc.tensor.transpose(tp[:], h_loc[:, w, :], ident[:])
                    hT = scr.tile([128, 128], f32, tag="hT")
                    nc.vector.tensor_copy(out=hT[:], in_=tp[:])
                    lp = psp.tile([128, 128], f32, tag="lp")
                    nc.tensor.matmul(lp[:], hT[:], LW[:], start=True, stop=True)
                    ep = psp.tile([128, 128], f32, tag="ep")
                    nc.tensor.matmul(ep[:], hT[:], EV[:], start=True, stop=True)
                    mk = scr.tile([128, 1], i8, tag="mk")
                    nc.vector.tensor_scalar(
                        out=mk[:], in0=S_all[:, w, 129:130], scalar1=0.0,
                        scalar2=None, op0=OP.is_gt,
                    )
                    nc.scalar.copy(hn[:, w, :], ep[:])
                    nc.vector.copy_predicated(
                        out=hn[:, w, :], mask=mk[:].to_broadcast([128, 128]),
                        data=lp[:],
                    )

                if stage in ("B1", "B"):
                    nc.sync.dma_start(
                        out=out_d[:].rearrange("(w p) d -> p w d", p=128),
                        in_=S_all[:, :, 0:128],
                    )
                    return nc
                # ---------------- phase D: per-node epilogue ---------------
                def B(tag):
                    return pc.tile([128, NW], f32, tag=tag, name=tag)[:]

                def TTb(dst, a, b, op):
                    nc.vector.tensor_tensor(out=dst, in0=a, in1=b, op=op)

                deg = S_all[:, :, 129]
                S0 = S_all[:, :, 128]
                q = B("Dq")
                TTb(q, norm_sb[:], deg, OP.mult)
                qq = B("Dqq")
                nc.vector.tensor_scalar(
                    out=qq, in0=q, scalar1=1e-6, scalar2=None, op0=OP.add
                )
                rq = B("Drq")
                nc.vector.reciprocal(rq, qq)
                fac = B("Dfac")
                TTb(fac, norm_sb[:], rq, OP.mult)
                mu0 = B("Dmu0")
                TTb(mu0, S0, fac, OP.mult)
                f2 = B("Df2")
                TTb(f2, fac, fac, OP.mult)
                s2 = B("Ds2")
                TTb(s2, s2raw[:], f2, OP.mult)
                m0s = B("Dm0s")
                TTb(m0s, mu0, mu0, OP.mult)
                mm = B("Dmm")
                nc.vector.tensor_scalar(
                    out=mm, in0=m0s, scalar1=-2.0, scalar2=None, op0=OP.mult
                )
                mink = B("Dmink")
                TTb(mink, s2, mm, OP.add)
                ab = B("Dab")
                nc.scalar.activation(ab, mink, AF.Abs)
                am = B("Dam")
                nc.vector.tensor_scalar(
                    out=am, in0=ab, scalar1=EPS, scalar2=None, op0=OP.max
                )
                sqm = B("Dsqm")
                nc.scalar.activation(sqm, am, AF.Sqrt)
                rr = B("Drr")
                nc.vector.reciprocal(rr, sqm)
                cf = B("Dcf")
                nc.vector.tensor_scalar(
                    out=cf, in0=rr, scalar1=1.0 / SC, scalar2=None, op0=OP.mult
                )
                c0 = B("Dc0")
                TTb(c0, mu0, cf, OP.mult)
                pd = B("Dpd")
                nc.vector.tensor_scalar(
                    out=pd, in0=c0, scalar1=SC, scalar2=1.0, op0=OP.mult, op1=OP.add
                )
                pdc = B("Dpdc")
                nc.vector.tensor_scalar(
                    out=pdc, in0=pd, scalar1=EPS, scalar2=None, op0=OP.max
                )
                rpd = B("Drpd")
                nc.vector.reciprocal(rpd, pdc)
                s_y = B("Dsy")
                TTb(s_y, cf, rpd, OP.mult)
                sp2 = B("Dsp2")
                TTb(sp2, s2, m0s, OP.subtract)
                y2 = B("Dy2")
                TTb(y2, s_y, s_y, OP.mult)
                ny2 = B("Dny2")
                TTb(ny2, y2, sp2, OP.mult)
                nyr = B("Dnyr")
                nc.scalar.activation(nyr, ny2, AF.Sqrt)
                ny = B("Dny")
                nc.vector.tensor_scalar(
                    out=ny, in0=nyr, scalar1=EPS, scalar2=None, op0=OP.max
                )
                v = B("Dv")
                nc.vector.tensor_scalar(
                    out=v, in0=ny, scalar1=SC, scalar2=1.0 - EPS,
                    op0=OP.mult, op1=OP.min,
                )
                la = B("Dla")
                nc.scalar.activation(la, v, AF.Ln, bias=1.0, scale=1.0)
                lb = B("Dlb")
                nc.scalar.activation(lb, v, AF.Ln, bias=1.0, scale=-1.0)
                df = B("Ddf")
                TTb(df, la, lb, OP.subtract)
                rny = B("Drny")
                nc.vector.reciprocal(rny, ny)
                k0 = B("Dk0")
                TTb(k0, df, rny, OP.mult)
                k1 = B("Dk1")
                TTb(k1, k0, s_y, OP.mult)
                k2 = B("Dk2")
                TTb(k2, k1, fac, OP.mult)
                hfac = pp.tile([128, NW], f32)
                nc.vector.tensor_scalar(
                    out=hfac[:], in0=k2, scalar1=0.5 / SC, scalar2=None, op0=OP.mult
                )
                for w in range(NW):
                    tmp = scr.tile([128, 128], f32, tag="d1")
                    hf_sl = hfac[:, w : w + 1]
                    hf_bc = bass.AP(
                        hf_sl.tensor, hf_sl.offset, [hf_sl.ap[0], [0, 128]]
                    )
                    nc.vector.tensor_tensor(
                        out=tmp[:], in0=S_all[:, w, 0:128], in1=hf_bc, op=OP.mult
                    )
                    nc.vector.tensor_scalar(
                        out=tmp[:], in0=tmp[:], scalar1=10.0, scalar2=-10.0,
                        op0=OP.min, op1=OP.max,
                    )
                    nc.vector.tensor_tensor(
                        out=hn[:, w, :], in0=tmp[:], in1=hn[:, w, :], op=OP.add
                    )
                    nc.vector.tensor_scalar(
                        out=hn[:, w, :], in0=hn[:, w, :], scalar1=10.0,
                        scalar2=-10.0, op0=OP.min, op1=OP.max,
                    )
                    sqd = scr.tile([128, 128], f32, tag="sqd")
                    nc.scalar.activation(
                        sqd[:], hn[:, w, :], AF.Square,
                        accum_out=ne2[:, w : w + 1],
                    )
                nnf = B("Dnnf")
                nc.scalar.activation(nnf, ne2[:], AF.Sqrt)
                nnc = B("Dnnc")
                nc.vector.tensor_scalar(
                    out=nnc, in0=nnf, scalar1=EPS, scalar2=None, op0=OP.max
                )
                thf = B("Dthf")
                nc.scalar.activation(thf, nnc, AF.Tanh, scale=SC)
                rnf = B("Drnf")
                nc.vector.reciprocal(rnf, nnc)
                sf0 = B("Dsf0")
                TTb(sf0, thf, rnf, OP.mult)
                sf = B("Dsf")
                nc.vector.tensor_scalar(
                    out=sf, in0=sf0, scalar1=1.0 / SC, scalar2=None, op0=OP.mult
                )
                for w in range(NW):
                    sf_sl = sf[:, w : w + 1]
                    sf_bc = bass.AP(
                        sf_sl.tensor, sf_sl.offset, [sf_sl.ap[0], [0, 128]]
                    )
                    nc.vector.tensor_tensor(
                        out=hn[:, w, :], in0=hn[:, w, :], in1=sf_bc, op=OP.mult
                    )
                nc.sync.dma_start(
                    out=out_d[:].rearrange("(w p) d -> p w d", p=128), in_=hn[:]
                )
    return nc


def get_program():
    global _PROGRAM
    if _PROGRAM is None:
        _PROGRAM = _build_program()
        _PROGRAM.compile()
    return _PROGRAM


# ------------------------------------------------------------ host wrapper
def _preprocess(h_hyper, weight, loop_weight, evolve_loop_weight, rel_emb,
                norm, src, dst, etype):
    wr = np.concatenate(
        [weight.reshape(R, D), rel_emb.reshape(R, D)], axis=1
    ).astype(np.float32)
    h_pad = np.zeros((NROT, D), np.float32)
    h_pad[:N] = h_hyper
    src = src.astype(np.int64)
    dst = dst.astype(np.int64)
    core = dst // NPC
    local = dst - core * NPC
    win = local // 128
    rel = (local % 128).astype(np.float32)

    def wrap_idx(a):
        # per-chunk 16-wrap: within chunk k, index i lives at
        # (partition i%16, col 72*k + i//16); replicate over 8 groups of 16.
        a2 = a.reshape(NCH, 72, 16).transpose(0, 2, 1).reshape(NCH, 16, 72)
        big = a2.transpose(1, 0, 2).reshape(16, IDXC)
        return np.tile(big, (8, 1)).astype(np.int16)

    in_maps = []
    for c in range(NCORES):
        m = core == c
        src_c, et_c, w_c, rel_c = src[m], etype[m], win[m], rel[m]
        order = np.argsort(w_c, kind="stable")
        src_c, et_c, w_c, rel_c = (
            src_c[order], et_c[order], w_c[order], rel_c[order],
        )
        counts = np.bincount(w_c, minlength=NW)
        if counts.max() > EPW:
            raise RuntimeError(
                f"window overflow: {counts.max()} edges > budget {EPW}"
            )
        offs = np.concatenate([[0], np.cumsum(counts)[:-1]])
        pos = w_c * EPW + (np.arange(len(w_c)) - offs[w_c])

        rot = (src_c - c * NPC) % NROT
        pair = np.zeros(ESLOT, np.int16)
        par = np.zeros(ESLOT, np.int8)
        etyp = np.zeros(ESLOT, np.int16)
        drelf = np.full(ESLOT, -1.0, np.float32)
        pair[pos] = (rot >> 1).astype(np.int16)
        par[pos] = (rot & 1).astype(np.int8)
        etyp[pos] = et_c.astype(np.int16)
        drelf[pos] = rel_c

        n_real = min(NPC, N - c * NPC)
        norm_c = np.ones((NPC, 1), np.float32)
        norm_c[:n_real] = norm[c * NPC : c * NPC + n_real].astype(np.float32)

        in_maps.append({
            "h_roll": np.roll(h_pad, -c * NPC, axis=0),
            "wr_tab": wr,
            "lw": loop_weight.astype(np.float32),
            "ev": evolve_loop_weight.astype(np.float32),
            "norm_c": norm_c,
            "idx_h": wrap_idx(pair),
            "idx_wr": wrap_idx(etyp),
            "par": par.reshape(NTILES, 128).T.copy(),
            "drel": drelf.reshape(NTILES, 128).T.copy(),
        })
    return in_maps


def run(inputs, trace=False, **kw):
    from concourse.bass_utils import run_bass_kernel_spmd

    nc = get_program()
    in_maps = _preprocess(**inputs)
    res = run_bass_kernel_spmd(nc, in_maps, list(range(NCORES)), trace=trace, **kw)
    parts = []
    for c in range(NCORES):
        n_real = min(NPC, N - c * NPC)
        parts.append(res.results[c]["out"][:n_real])
    out = np.concatenate(parts, axis=0)
    return out, res


def kernel(**inputs) -> np.ndarray:
    out, _ = run(inputs)
    return out



# revision 24
# speedup vs baseline: 1.5057x; 1.5057x over previous
"""Trainium2 Bass kernel for a Lorentz RGCN message-passing layer.

Strategy (8 NeuronCores, SPMD, no collectives):
  - Nodes are range-partitioned by destination: core c owns 6272 dst nodes.
    Each core processes all edges whose dst it owns and writes a disjoint
    slice of the output.
  - Within a core, its 6272 nodes are PERMUTED into 49 windows of 128 by
    LPT (longest-processing-time) bin packing on in-degree, so every
    window holds <= 17*128 = 2176 edges (vs 18 tiles for the naive
    contiguous split).  One dma_gather of 2176 pair-indices per window.
  - Since NUM_BASES == D (SI=SO=1) the relation transform is elementwise:
    msg = h_tangent[src] * weight[etype] + rel_emb[etype].  The
    weight/rel_emb rows are expanded per edge-slot ON THE HOST into a
    [128, NTILES, 256] f16 table streamed with plain contiguous DMA
    (no per-edge gather descriptors for the tables).
  - h_tangent lives in DRAM as a f16 table; rows are fetched in PAIRS
    (pair index < 25088 fits the gather's int16 index ucode) and the
    right half is selected by parity via copy_predicated.
  - Per-edge exp0/to_lorentz scalars are batched per GROUP of 7 windows
    ([128, 119] ops) to amortize per-instruction overhead.
  - Segment sums: TensorEngine one-hot matmuls (f16) into a PSUM
    [128 nodes x 130] accumulator per window; per-edge centroid weight
    is folded into the one-hot.
  - Self-loop matmuls use a DMA-transposed f16 copy of the core's own
    h_tangent block; per-node epilogue is batched per group.
"""

import sys

sys.path.insert(0, "/opt/trn_rl_repo")

import numpy as np

import concourse.bass as bass
import concourse.bacc as bacc
import concourse.mybir as mybir
from concourse.tile import TileContext

# ---------------------------------------------------------------- constants
NCORES = 8
N = 50000
E = 800000
D = 128
R = 230
C = 0.01
SC = 0.1  # sqrt(C)
EPS = 1e-7

NPC = 6272                 # nodes per core = 49 windows * 128
NW = 49                    # windows per core
TPW = 17                   # tiles per window (LPT-balanced, max load ~2054)
EPW = TPW * 128            # 2176 edge slots per window
NTILES = NW * TPW          # 833
ESLOT = NTILES * 128       # 106624 edge slots per core
NROT = NCORES * NPC        # 50176 rows in the (rolled, padded) h table
GRP = 7                    # windows per group (chain batching)
NGRP = NW // GRP           # 7
IDXW = EPW // 16           # 136 index columns per window
IDXC = NW * IDXW           # 6664

f32 = mybir.dt.float32
f16 = mybir.dt.float16
i16 = mybir.dt.int16
i32 = mybir.dt.int32
i8 = mybir.dt.int8
OP = mybir.AluOpType
AF = mybir.ActivationFunctionType

SUP = 14                   # rows-per-partition per phase-A supertile
NSUP = NROT // (SUP * 128)  # 28


# ------------------------------------------------------------ device program
_PROGRAM = None


def _build_program():
    nc = bacc.Bacc("TRN2", target_bir_lowering=False, debug=False)

    h_roll = nc.declare_dram_parameter("h_roll", [NROT, D], f16, isOutput=False)
    wr_e = nc.declare_dram_parameter("wr_e", [128, NTILES, 2 * D], f16, isOutput=False)
    lw_d = nc.declare_dram_parameter("lw", [D, D], f16, isOutput=False)
    ev_d = nc.declare_dram_parameter("ev", [D, D], f16, isOutput=False)
    norm_d = nc.declare_dram_parameter("norm_c", [NPC, 1], f32, isOutput=False)
    deg_d = nc.declare_dram_parameter("deg_c", [NPC, 1], f32, isOutput=False)
    idxh_d = nc.declare_dram_parameter("idx_h", [128, IDXC], i16, isOutput=False)
    par_d = nc.declare_dram_parameter("par", [128, NTILES], i8, isOutput=False)
    drel_d = nc.declare_dram_parameter("drel", [128, NTILES], f16, isOutput=False)
    out_d = nc.declare_dram_parameter("out", [NPC, D], f32, isOutput=True)
    htab = nc.dram_tensor("htab", [NROT, D], f16)

    with TileContext(nc) as tc:
        with (
            tc.tile_pool(name="persist", bufs=1) as pp,
            tc.tile_pool(name="consts", bufs=1) as cp,
        ):
            hT = pp.tile([128, NPC], f16)          # h_tangent^T of own nodes
            par_sb = pp.tile([128, NTILES], i8)
            drel_sb = pp.tile([128, NTILES], f16)
            norm_sb = pp.tile([128, NW], f32)
            deg_sb = pp.tile([128, NW], f32)

            LW = cp.tile([128, D], f16)
            EV = cp.tile([128, D], f16)
            iota_bf = cp.tile([128, 128], f16)
            iota_i = cp.tile([128, 128], i32)

            nc.sync.dma_start(out=par_sb[:], in_=par_d[:])
            nc.sync.dma_start(out=drel_sb[:], in_=drel_d[:])
            nc.sync.dma_start(
                out=norm_sb[:], in_=norm_d[:].rearrange("(w p) o -> p (w o)", p=128)
            )
            nc.sync.dma_start(
                out=deg_sb[:], in_=deg_d[:].rearrange("(w p) o -> p (w o)", p=128)
            )
            nc.sync.dma_start(out=LW[:], in_=lw_d[:])
            nc.sync.dma_start(out=EV[:], in_=ev_d[:])
            nc.gpsimd.iota(iota_i[:], pattern=[[1, 128]], base=0, channel_multiplier=0)
            nc.vector.tensor_copy(out=iota_bf[:], in_=iota_i[:])

            # ---------------- phase A: h_tangent table (log0 of h_roll) ----
            # (p t) layout: supertile s covers rows [s*1792, (s+1)*1792),
            # partition p holds rows s*1792 + p*14 .. +13 -> 3.5KB DMA descs.
            with tc.tile_pool(name="phA", bufs=3) as pa:
                for s in range(NSUP):
                    r0 = s * SUP * 128
                    xin = pa.tile([128, SUP, D], f16, tag="xin")
                    nc.sync.dma_start(
                        out=xin[:],
                        in_=h_roll[r0 : r0 + SUP * 128, :].rearrange(
                            "(p t) d -> p t d", t=SUP
                        ),
                    )
                    sqv = pa.tile([128, SUP, D], f16, tag="sqv")
                    nc.scalar.activation(sqv[:], xin[:], AF.Square)
                    n2 = pa.tile([128, SUP], f32, tag="n2")
                    nc.vector.reduce_sum(out=n2[:], in_=sqv[:], axis=mybir.AxisListType.X)
                    nr = pa.tile([128, SUP], f32, tag="nr")
                    nc.scalar.activation(nr[:], n2[:], AF.Sqrt)
                    nn = pa.tile([128, SUP], f32, tag="nn")
                    nc.vector.tensor_scalar(
                        out=nn[:], in0=nr[:], scalar1=EPS, scalar2=None, op0=OP.max
                    )
                    v = pa.tile([128, SUP], f32, tag="v")
                    nc.vector.tensor_scalar(
                        out=v[:], in0=nn[:], scalar1=SC, scalar2=1.0 - EPS,
                        op0=OP.mult, op1=OP.min,
                    )
                    la = pa.tile([128, SUP], f32, tag="la")
                    nc.scalar.activation(la[:], v[:], AF.Ln, bias=1.0, scale=1.0)
                    lb = pa.tile([128, SUP], f32, tag="lb")
                    nc.scalar.activation(lb[:], v[:], AF.Ln, bias=1.0, scale=-1.0)
                    df = pa.tile([128, SUP], f32, tag="df")
                    nc.vector.tensor_tensor(
                        out=df[:], in0=la[:], in1=lb[:], op=OP.subtract
                    )
                    rn = pa.tile([128, SUP], f32, tag="rn")
                    nc.vector.reciprocal(rn[:], nn[:])
                    scl = pa.tile([128, SUP], f32, tag="scl")
                    nc.vector.scalar_tensor_tensor(
                        out=scl[:], in0=df[:], scalar=0.5 / SC, in1=rn[:],
                        op0=OP.mult, op1=OP.mult,
                    )
                    hts = pa.tile([128, SUP, D], f16, tag="hts")
                    scl_bc = bass.AP(
                        scl.tensor, scl.offset, [scl.ap[0], scl.ap[1], [0, D]]
                    )
                    nc.vector.tensor_tensor(
                        out=hts[:], in0=xin[:], in1=scl_bc, op=OP.mult
                    )
                    nc.sync.dma_start(
                        out=htab[r0 : r0 + SUP * 128, :].rearrange(
                            "(p t) d -> p t d", t=SUP
                        ),
                        in_=hts[:],
                    )

            tc.strict_bb_all_engine_barrier()
            # transposed copy of own nodes' h_tangent for self-loop matmuls
            nc.sync.dma_start_transpose(hT[:], htab[0:NPC, :])

            # ---------------- phase B/C/D: edges, segments, epilogue -------
            htab_pairs = htab[:].rearrange("(a b) d -> a (b d)", b=2)
            nreg = nc.gpsimd.to_reg(EPW)
            with (
                tc.tile_pool(name="pid", bufs=3) as pid,
                tc.tile_pool(name="phb", bufs=2) as phb,
                tc.tile_pool(name="pwr", bufs=2) as pwr,
                tc.tile_pool(name="scr", bufs=2) as scr,
                tc.tile_pool(name="pg", bufs=2) as pg,
                tc.tile_pool(name="pc", bufs=2) as pc,
                tc.tile_pool(name="psum", bufs=2, space="PSUM") as psp,
            ):
                for g in range(NGRP):
                    n2g = pg.tile([128, GRP * TPW], f32, tag="n2g")
                    Sg = pg.tile([128, GRP, 129], f32, tag="Sg")
                    hng = pg.tile([128, GRP, D], f32, tag="hng")
                    s2r = pg.tile([128, GRP], f32, tag="s2r")
                    rhs_list = []
                    for j in range(GRP):
                        w = g * GRP + j
                        idx_t = pid.tile([128, IDXW], i16, tag="idx")
                        nc.sync.dma_start(
                            out=idx_t[:], in_=idxh_d[:, IDXW * w : IDXW * (w + 1)]
                        )
                        hb = phb.tile([128, TPW, 2 * D], f16, tag="hb")
                        nc.gpsimd.dma_gather(
                            out_ap=hb[:], in_ap=htab_pairs, idxs_ap=idx_t[:],
                            num_idxs=EPW, num_idxs_reg=nreg, elem_size=2 * D,
                            single_packet=False,
                        )
                        wrb = pwr.tile([128, TPW, 2 * D], f16, tag="wrb")
                        nc.sync.dma_start(
                            out=wrb[:], in_=wr_e[:, TPW * w : TPW * (w + 1), :]
                        )
                        rhs_w = pg.tile([128, TPW, 129], f16, tag=f"rhs{j}")
                        rhs_list.append(rhs_w)
                        msg = rhs_w[:, :, 0:128]
                        nc.scalar.copy(msg, hb[:, :, 0:128])
                        nc.vector.copy_predicated(
                            out=msg,
                            mask=par_sb[:, TPW * w : TPW * (w + 1)].to_broadcast(
                                [128, TPW, 128]
                            ),
                            data=hb[:, :, 128:256],
                        )
                        nc.vector.tensor_tensor(
                            out=msg, in0=msg, in1=wrb[:, :, 0:128], op=OP.mult
                        )
                        nc.vector.tensor_tensor(
                            out=msg, in0=msg, in1=wrb[:, :, 128:256], op=OP.add
                        )
                        sqv = scr.tile([128, TPW, D], f16, tag="sqv")
                        nc.scalar.activation(sqv[:], msg, AF.Square)
                        nc.vector.reduce_sum(
                            out=n2g[:, TPW * j : TPW * (j + 1)], in_=sqv[:],
                            axis=mybir.AxisListType.X,
                        )

                    # ---- batched per-edge chain on [128, 119] -------------
                    def PCT(tag):
                        return pc.tile([128, GRP * TPW], f32, tag=tag, name=tag)[:]

                    def TS(dst, src, s1, s2=None, o0=OP.mult, o1=None):
                        if o1 is None:
                            nc.vector.tensor_scalar(
                                out=dst, in0=src, scalar1=s1, scalar2=None, op0=o0
                            )
                        else:
                            nc.vector.tensor_scalar(
                                out=dst, in0=src, scalar1=s1, scalar2=s2,
                                op0=o0, op1=o1,
                            )

                    def TT(dst, a, b, op):
                        nc.vector.tensor_tensor(out=dst, in0=a, in1=b, op=op)

                    nraw = PCT("nraw")
                    nc.scalar.activation(nraw, n2g[:], AF.Sqrt)
                    nn = PCT("nn")
                    TS(nn, nraw, EPS, o0=OP.max)
                    th = PCT("th")
                    nc.scalar.activation(th, nn, AF.Tanh, scale=SC)
                    t2 = PCT("t2")
                    TT(t2, th, th, OP.mult)
                    dn = PCT("dn")
                    TS(dn, t2, -1.0, 1.0, OP.mult, OP.add)
                    dnc = PCT("dnc")
                    TS(dnc, dn, EPS, o0=OP.max)
                    rd = PCT("rd")
                    nc.vector.reciprocal(rd, dnc)
                    rn = PCT("rn")
                    nc.vector.reciprocal(rn, nn)
                    a1 = PCT("a1")
                    nc.vector.scalar_tensor_tensor(
                        out=a1, in0=th, scalar=2.0 / SC, in1=rn,
                        op0=OP.mult, op1=OP.mult,
                    )
                    sxi = PCT("sxi")
                    TT(sxi, a1, rd, OP.mult)
                    sxi_bf = pc.tile([128, GRP * TPW], f16, tag="sxibf")
                    nc.vector.tensor_copy(out=sxi_bf[:], in_=sxi)
                    # inv from the ROUNDED sxi so that sxi_f16 * dxp == dx
                    # exactly (the one-hot carries sxi_f16).
                    sxi_r = PCT("sxir")
                    nc.vector.tensor_copy(out=sxi_r, in_=sxi_bf[:])
                    inv = PCT("inv")
                    nc.vector.reciprocal(inv, sxi_r)
                    # dx = x0 - 1/SC = 2*t2/(SC*dn): the deviation keeps full
                    # relative precision through the f16 matmul column.
                    dx = PCT("dx")
                    nc.vector.scalar_tensor_tensor(
                        out=dx, in0=t2, scalar=2.0 / SC, in1=rd,
                        op0=OP.mult, op1=OP.mult,
                    )
                    dxp = PCT("dxp")
                    TT(dxp, dx, inv, OP.mult)

                    # ---- per-window: one-hot, segment matmuls, phase C ----
                    for j in range(GRP):
                        w = g * GRP + j
                        rhs_w = rhs_list[j]
                        nc.scalar.copy(
                            rhs_w[:, :, 128], dxp[:, TPW * j : TPW * (j + 1)]
                        )
                        selc = scr.tile([128, TPW, 128], f16, tag="selc")
                        iota_bc = bass.AP(
                            iota_bf.tensor, iota_bf.offset,
                            [iota_bf.ap[0], [0, TPW], iota_bf.ap[1]],
                        )
                        nc.vector.tensor_tensor(
                            out=selc[:], in0=iota_bc,
                            in1=drel_sb[:, TPW * w : TPW * (w + 1)].to_broadcast(
                                [128, TPW, 128]
                            ),
                            op=OP.is_equal,
                        )
                        nc.vector.tensor_tensor(
                            out=selc[:], in0=selc[:],
                            in1=sxi_bf[:, TPW * j : TPW * (j + 1)].to_broadcast(
                                [128, TPW, 128]
                            ),
                            op=OP.mult,
                        )
                        ps = psp.tile([128, 129], f32, tag="ps")
                        for t in range(TPW):
                            nc.tensor.matmul(
                                ps[:], selc[:, t, :], rhs_w[:, t, :],
                                start=(t == 0), stop=(t == TPW - 1),
                            )
                        # phase C
                        nc.scalar.copy(Sg[:, j, :], ps[:])
                        sq2 = scr.tile([128, 128], f16, tag="sq2")
                        nc.scalar.activation(
                            sq2[:], Sg[:, j, 0:128], AF.Square,
                            accum_out=s2r[:, j : j + 1],
                        )
                        lp = psp.tile([128, 128], f32, tag="lp")
                        nc.tensor.matmul(
                            lp[:], hT[:, 128 * w : 128 * (w + 1)], LW[:],
                            start=True, stop=True,
                        )
                        ep = psp.tile([128, 128], f32, tag="ep")
                        nc.tensor.matmul(
                            ep[:], hT[:, 128 * w : 128 * (w + 1)], EV[:],
                            start=True, stop=True,
                        )
                        mk = scr.tile([128, 1], i8, tag="mk")
                        nc.vector.tensor_scalar(
                            out=mk[:], in0=deg_sb[:, w : w + 1], scalar1=0.0,
                            scalar2=None, op0=OP.is_gt,
                        )
                        nc.scalar.copy(hng[:, j, :], ep[:])
                        nc.vector.copy_predicated(
                            out=hng[:, j, :], mask=mk[:].to_broadcast([128, 128]),
                            data=lp[:],
                        )

                    # ---- phase D: per-node epilogue for the group ---------
                    def B(tag):
                        return pc.tile([128, GRP], f32, tag=tag, name=tag)[:]

                    nrm = norm_sb[:, g * GRP : (g + 1) * GRP]
                    deg = deg_sb[:, g * GRP : (g + 1) * GRP]
                    Sdx = Sg[:, :, 128]
                    q = B("Dq")
                    TT(q, nrm, deg, OP.mult)
                    qq = B("Dqq")
                    TS(qq, q, 1e-6, o0=OP.add)
                    rq = B("Drq")
                    nc.vector.reciprocal(rq, qq)
                    fac = B("Dfac")
                    TT(fac, nrm, rq, OP.mult)
                    S0 = B("DS0")
                    nc.vector.scalar_tensor_tensor(
                        out=S0, in0=deg, scalar=1.0 / SC, in1=Sdx,
                        op0=OP.mult, op1=OP.add,
                    )
                    mu0 = B("Dmu0")
                    TT(mu0, S0, fac, OP.mult)
                    f2 = B("Df2")
                    TT(f2, fac, fac, OP.mult)
                    s0sq = B("Ds0sq")
                    TT(s0sq, S0, S0, OP.mult)
                    s2a = B("Ds2a")
                    TT(s2a, s2r[:], s0sq, OP.add)
                    s2 = B("Ds2")
                    TT(s2, s2a, f2, OP.mult)
                    m0s = B("Dm0s")
                    TT(m0s, mu0, mu0, OP.mult)
                    mink = B("Dmink")
                    nc.vector.scalar_tensor_tensor(
                        out=mink, in0=m0s, scalar=-2.0, in1=s2,
                        op0=OP.mult, op1=OP.add,
                    )
                    ab = B("Dab")
                    nc.scalar.activation(ab, mink, AF.Abs)
                    am = B("Dam")
                    TS(am, ab, EPS, o0=OP.max)
                    sqm = B("Dsqm")
                    nc.scalar.activation(sqm, am, AF.Sqrt)
                    rr = B("Drr")
                    nc.vector.reciprocal(rr, sqm)
                    c0 = B("Dc0")
                    nc.vector.scalar_tensor_tensor(
                        out=c0, in0=mu0, scalar=1.0 / SC, in1=rr,
                        op0=OP.mult, op1=OP.mult,
                    )
                    pd = B("Dpd")
                    TS(pd, c0, SC, 1.0, OP.mult, OP.add)
                    pdc = B("Dpdc")
                    TS(pdc, pd, EPS, o0=OP.max)
                    rpd = B("Drpd")
                    nc.vector.reciprocal(rpd, pdc)
                    s_y = B("Dsy")
                    nc.vector.scalar_tensor_tensor(
                        out=s_y, in0=rr, scalar=1.0 / SC, in1=rpd,
                        op0=OP.mult, op1=OP.mult,
                    )
                    sp2 = B("Dsp2")
                    TT(sp2, s2, m0s, OP.subtract)
                    y2 = B("Dy2")
                    TT(y2, s_y, s_y, OP.mult)
                    ny2 = B("Dny2")
                    TT(ny2, y2, sp2, OP.mult)
                    nyr = B("Dnyr")
                    nc.scalar.activation(nyr, ny2, AF.Sqrt)
                    ny = B("Dny")
                    TS(ny, nyr, EPS, o0=OP.max)
                    v = B("Dv")
                    TS(v, ny, SC, 1.0 - EPS, OP.mult, OP.min)
                    la = B("Dla")
                    nc.scalar.activation(la, v, AF.Ln, bias=1.0, scale=1.0)
                    lb = B("Dlb")
                    nc.scalar.activation(lb, v, AF.Ln, bias=1.0, scale=-1.0)
                    df = B("Ddf")
                    TT(df, la, lb, OP.subtract)
                    rny = B("Drny")
                    nc.vector.reciprocal(rny, ny)
                    t1 = B("Dt1")
                    nc.vector.scalar_tensor_tensor(
                        out=t1, in0=df, scalar=0.5 / SC, in1=rny,
                        op0=OP.mult, op1=OP.mult,
                    )
                    k1 = B("Dk1")
                    TT(k1, t1, s_y, OP.mult)
                    hfac = B("Dhfac")
                    TT(hfac, k1, fac, OP.mult)

                    # big [128, GRP, 128] ops
                    tmp = scr.tile([128, GRP, D], f32, tag="Dtmp")
                    nc.vector.tensor_tensor(
                        out=tmp[:], in0=Sg[:, :, 0:128],
                        in1=hfac.to_broadcast([128, GRP, 128]), op=OP.mult
                    )
                    nc.vector.tensor_scalar(
                        out=tmp[:], in0=tmp[:], scalar1=10.0, scalar2=-10.0,
                        op0=OP.min, op1=OP.max,
                    )
                    nc.vector.tensor_tensor(
                        out=hng[:], in0=tmp[:], in1=hng[:], op=OP.add
                    )
                    nc.vector.tensor_scalar(
                        out=hng[:], in0=hng[:], scalar1=10.0, scalar2=-10.0,
                        op0=OP.min, op1=OP.max,
                    )
                    sqd = scr.tile([128, GRP, D], f16, tag="Dsqd")
                    nc.scalar.activation(sqd[:], hng[:], AF.Square)
                    ne2 = B("Dne2")
                    nc.vector.reduce_sum(
                        out=ne2, in_=sqd[:], axis=mybir.AxisListType.X
                    )
                    nnf = B("Dnnf")
                    nc.scalar.activation(nnf, ne2, AF.Sqrt)
                    nnc = B("Dnnc")
                    TS(nnc, nnf, EPS, o0=OP.max)
                    thf = B("Dthf")
                    nc.scalar.activation(thf, nnc, AF.Tanh, scale=SC)
                    rnf = B("Drnf")
                    nc.vector.reciprocal(rnf, nnc)
                    sf = B("Dsf")
                    nc.vector.scalar_tensor_tensor(
                        out=sf, in0=thf, scalar=1.0 / SC, in1=rnf,
                        op0=OP.mult, op1=OP.mult,
                    )
                    nc.vector.tensor_tensor(
                        out=hng[:], in0=hng[:],
                        in1=sf.to_broadcast([128, GRP, 128]), op=OP.mult
                    )
                    r0 = g * GRP * 128
                    nc.sync.dma_start(
                        out=out_d[r0 : r0 + GRP * 128, :].rearrange(
                            "(w p) d -> p w d", p=128
                        ),
                        in_=hng[:],
                    )
    return nc


def get_program():
    global _PROGRAM
    if _PROGRAM is None:
        _PROGRAM = _build_program()
        _PROGRAM.compile()
    return _PROGRAM


# ------------------------------------------------------------ host wrapper
def _lpt_permute(deg):
    """Assign NPC nodes to NW capacity-128 windows, balancing degree sums.
    Returns p2n: position -> original local node (positions are
    window-major: position = window*128 + lane)."""
    import heapq

    order = np.argsort(-deg, kind="stable")
    heap = [(0, w) for w in range(NW)]
    heapq.heapify(heap)
    members = [[] for _ in range(NW)]
    for n in order:
        while True:
            load, w = heapq.heappop(heap)
            if len(members[w]) < 128:
                break
        members[w].append(n)
        if len(members[w]) < 128:
            heapq.heappush(heap, (load + int(deg[n]), w))
    p2n = np.concatenate(
        [np.array(m, dtype=np.int64) for m in members]
    )
    return p2n


def _preprocess(h_hyper, weight, loop_weight, evolve_loop_weight, rel_emb,
                norm, src, dst, etype):
    wrcat = np.concatenate(
        [weight.reshape(R, D), rel_emb.reshape(R, D)], axis=1
    ).astype(np.float32)
    h_pad = np.zeros((NROT, D), np.float32)
    h_pad[:N] = h_hyper
    src = src.astype(np.int64)
    dst = dst.astype(np.int64)
    core = dst // NPC

    def wrap_idx(a):
        # per-window 16-wrap: within window w, index i lives at
        # (partition i%16, col IDXW*w + i//16); replicated over 8 groups.
        a2 = a.reshape(NW, IDXW, 16).transpose(0, 2, 1).reshape(NW, 16, IDXW)
        big = a2.transpose(1, 0, 2).reshape(16, IDXC)
        return np.tile(big, (8, 1)).astype(np.int16)

    in_maps = []
    perms = []
    for c in range(NCORES):
        m = core == c
        src_c, et_c = src[m], etype[m]
        d_loc = dst[m] - c * NPC
        deg = np.bincount(d_loc, minlength=NPC)
        p2n = _lpt_permute(deg)
        n2p = np.empty(NPC, np.int64)
        n2p[p2n] = np.arange(NPC)
        perms.append(p2n)

        pos_node = n2p[d_loc]              # permuted position of each dst
        win = pos_node >> 7
        lane = (pos_node & 127).astype(np.float32)

        order = np.argsort(win, kind="stable")
        src_c, et_c, win, lane = src_c[order], et_c[order], win[order], lane[order]
        counts = np.bincount(win, minlength=NW)
        if counts.max() > EPW:
            raise RuntimeError(
                f"window overflow: {counts.max()} edges > budget {EPW}"
            )
        offs = np.concatenate([[0], np.cumsum(counts)[:-1]])
        slot = win * EPW + (np.arange(len(win)) - offs[win])

        # table-row map: own nodes sit (permuted) in rows [0, NPC); the
        # rest of the globe follows in rolled order.
        glob2row = np.empty(NROT, np.int64)
        glob2row[(c * NPC + p2n) % NROT] = np.arange(NPC)
        rest_glob = (np.arange(NPC, NROT) + c * NPC) % NROT
        glob2row[rest_glob] = np.arange(NPC, NROT)
        rot = glob2row[src_c]

        pair = np.zeros(ESLOT, np.int64)
        par = np.zeros(ESLOT, np.int8)
        drelf = np.full(ESLOT, -1.0, np.float32)
        pair[slot] = rot >> 1
        par[slot] = (rot & 1).astype(np.int8)
        drelf[slot] = lane
        wr_s = np.zeros((ESLOT, 2 * D), np.float32)
        wr_s[slot] = wrcat[et_c]

        # h table in (rolled + own-permuted) order
        rowsrc = np.empty(NROT, np.int64)
        rowsrc[0:NPC] = (c * NPC + p2n) % NROT
        rowsrc[NPC:] = rest_glob
        h_roll = h_pad[rowsrc].astype(np.float16)

        n_real = min(NPC, N - c * NPC)
        norm_full = np.ones(NPC, np.float32)
        norm_full[:n_real] = norm[c * NPC : c * NPC + n_real, 0].astype(np.float32)
        norm_c = norm_full[p2n].reshape(NPC, 1)
        deg_c = deg.astype(np.float32)[p2n].reshape(NPC, 1)

        in_maps.append({
            "h_roll": h_roll,
            "wr_e": wr_s.reshape(NTILES, 128, 2 * D).transpose(1, 0, 2)
                        .astype(np.float16),
            "lw": loop_weight.astype(np.float16),
            "ev": evolve_loop_weight.astype(np.float16),
            "norm_c": norm_c,
            "deg_c": deg_c,
            "idx_h": wrap_idx(pair),
            "par": par.reshape(NTILES, 128).T.copy(),
            "drel": drelf.reshape(NTILES, 128).T.astype(np.float16),
        })
    return in_maps, perms


def run(inputs, trace=False, **kw):
    from concourse.bass_utils import run_bass_kernel_spmd

    nc = get_program()
    in_maps, perms = _preprocess(**inputs)
    res = run_bass_kernel_spmd(nc, in_maps, list(range(NCORES)), trace=trace, **kw)
    out = np.empty((N, D), np.float32)
    for c in range(NCORES):
        n_real = min(NPC, N - c * NPC)
        o = res.results[c]["out"]            # rows are permuted positions
        p2n = perms[c]
        keep = p2n < n_real
        out[c * NPC + p2n[keep]] = o[keep]
    return out, res


def kernel(**inputs) -> np.ndarray:
    out, _ = run(inputs)
    return out
